# revision 28
# baseline (speedup 1.0000x reference)
"""Trainium2 Bass kernel for nn_MultiHeadAttention_62319975465542.

Tensor-parallel over heads (Megatron-style): 32 heads sharded 4-per-core
across 8 NeuronCores. Each core computes its heads' QKV projections,
attention, and a partial output projection; the host sums the 8 partials
(the all-reduce after Wo) and adds the output bias.

Reference layout note: Q = (X @ Wq.T + b).reshape(L, D_HEAD, NUM_HEADS),
so head h owns interleaved feature columns {d*32 + h : d in 0..63}. The
host pre-gathers those columns into contiguous per-core blocks.

v3: all matmul inputs in bf16 (fast weight load, halved DMA), X^T fully
SBUF-resident, and emission ordered so ScalarE exp for one l-block
overlaps TensorE work of the next (Q(lb0) first, then K/V, then
attention blocks). Softmax denominators ride in the A-V matmul as a
17th "ones" V-column ([V_h | 1], M=65); normalization is reciprocal +
gpsimd partition-broadcast + one vector multiply. PSUM stays fp32.
"""

import numpy as np
import ml_dtypes

import concourse.bass as bass
import concourse.tile as tile
import concourse.mybir as mybir
from concourse import bacc

F32 = mybir.dt.float32
BF16 = mybir.dt.bfloat16
Identity = mybir.ActivationFunctionType.Identity
Exp = mybir.ActivationFunctionType.Exp
MULT = mybir.AluOpType.mult
ADD = mybir.AluOpType.add

L = 2048          # sequence length
D = 2048          # d_model
NH = 32           # total heads
DH = 64           # head dim
NCORES = 8
HPC = NH // NCORES   # heads per core = 4
JC = HPC * DH        # per-core projected width = 256
LB = 512             # l-block width
NLB = L // LB        # 4
KO = D // 128        # 16 contraction chunks
MC = L // 128        # 16 key chunks


def build_program():
    nc = bacc.Bacc("TRN2", target_bir_lowering=False, debug=False)

    xt_d = nc.dram_tensor("XT", (D, L), BF16, kind="ExternalInput")
    wq_d = nc.dram_tensor("WQ", (128, KO, JC), BF16, kind="ExternalInput")
    wk_d = nc.dram_tensor("WK", (128, KO, JC), BF16, kind="ExternalInput")
    wv_d = nc.dram_tensor("WV", (128, KO, JC), BF16, kind="ExternalInput")
    wo_d = nc.dram_tensor("WO", (128, 2, D), BF16, kind="ExternalInput")
    bq_d = nc.dram_tensor("BQ", (128, 2), F32, kind="ExternalInput")
    bk_d = nc.dram_tensor("BK", (128, 2), F32, kind="ExternalInput")
    bv_d = nc.dram_tensor("BV", (1, JC), BF16, kind="ExternalInput")
    ones_d = nc.dram_tensor("ONES", (1, 128), BF16, kind="ExternalInput")
    y_d = nc.dram_tensor("Y", (L, D), F32, kind="ExternalOutput")

    with tile.TileContext(nc) as tc, nc.allow_low_precision(
            reason="bf16 activations are within tolerance for this op"):
        with (
            tc.tile_pool(name="const", bufs=1) as cp,
            tc.tile_pool(name="epool", bufs=4) as epool,
            tc.tile_pool(name="norm", bufs=2) as normp,
            tc.tile_pool(name="ysb", bufs=3) as ypool,
            tc.tile_pool(name="scps", bufs=2, space="PSUM") as scps,
            tc.tile_pool(name="accps", bufs=4, space="PSUM") as accps,
        ):
            wq_sb = cp.tile((128, KO, JC), BF16)
            wk_sb = cp.tile((128, KO, JC), BF16)
            wv_sb = cp.tile((128, KO, JC), BF16)
            wo_sb = cp.tile((128, 2, D), BF16)
            bq_sb = cp.tile((128, 2), F32)
            bk_sb = cp.tile((128, 2), F32)
            bv_sb = cp.tile((1, JC), BF16)
            ones_sb = cp.tile((1, 128), BF16)
            xt_sb = cp.tile((128, KO, L), BF16)
            nc.sync.dma_start(wq_sb[:], wq_d[:])
            nc.sync.dma_start(wk_sb[:], wk_d[:])
            nc.sync.dma_start(wv_sb[:], wv_d[:])
            nc.sync.dma_start(wo_sb[:], wo_d[:])
            nc.sync.dma_start(bq_sb[:], bq_d[:])
            nc.sync.dma_start(bk_sb[:], bk_d[:])
            nc.sync.dma_start(bv_sb[:], bv_d[:])
            nc.sync.dma_start(ones_sb[:], ones_d[:])

            qt_sb = [cp.tile((128, L), BF16, name=f"qt{p}") for p in range(2)]
            kt_sb = [cp.tile((128, L), BF16, name=f"kt{p}") for p in range(2)]
            v_sb = cp.tile((128, MC, HPC * 65), BF16)
            nc.vector.memset(v_sb[:], 1.0)
            ot_sb = [cp.tile((128, L), BF16, name=f"ot{p}") for p in range(2)]

            # X^T loads: lb-major so Q(lb0)/K(lb0) unblock after 16 DMAs
            for lb in range(NLB):
                for ko in range(KO):
                    nc.sync.dma_start(
                        xt_sb[:, ko, lb * LB:(lb + 1) * LB],
                        xt_d[ko * 128:(ko + 1) * 128, lb * LB:(lb + 1) * LB],
                    )

            def proj_qk(w_sb, b_sb, dst, lb):
                """(j, l) layout projection with per-partition bias via DVE."""
                for jc in range(2):
                    ps = accps.tile((128, LB), F32, name="acc_ps")
                    for ko in range(KO):
                        nc.tensor.matmul(
                            ps[:],
                            w_sb[:, ko, jc * 128:(jc + 1) * 128],
                            xt_sb[:, ko, lb * LB:(lb + 1) * LB],
                            start=(ko == 0), stop=(ko == KO - 1),
                        )
                    nc.vector.tensor_scalar(
                        dst[jc][:, lb * LB:(lb + 1) * LB], ps[:],
                        b_sb[:, jc:jc + 1], None, ADD,
                    )

            def proj_v(lb):
                """V in (l, j) layout; bias via K=1 ones-row matmul."""
                for lt in range(4):
                    vp = accps.tile((128, LB), F32, name="acc_ps")
                    for ko in range(KO):
                        nc.tensor.matmul(
                            vp[:, 0:JC],
                            xt_sb[:, ko, lb * LB + lt * 128:lb * LB + (lt + 1) * 128],
                            wv_sb[:, ko, :],
                            start=(ko == 0), stop=False,
                        )
                    nc.tensor.matmul(
                        vp[:, 0:JC], ones_sb[0:1, :], bv_sb[0:1, :],
                        start=False, stop=True,
                    )
                    nc.vector.tensor_copy(
                        v_sb[:, lb * 4 + lt, :].rearrange(
                            "p (h e) -> p h e", h=HPC)[:, :, 0:DH],
                        vp[:, 0:JC].rearrange("p (h d) -> p h d", h=HPC))

            def attn_pair_start():
                ava = accps.tile((128, LB), F32, name="acc_ps")
                avb = accps.tile((128, LB), F32, name="acc_ps")
                return ava, avb

            def attn_chunk(lb, p, m, ava, avb):
                lsl = slice(lb * LB, (lb + 1) * LB)
                msl = slice(m * 128, (m + 1) * 128)
                sc = scps.tile((128, 2 * LB), F32, name="sc_ps")
                nc.tensor.matmul(
                    sc[:, 0:LB],
                    kt_sb[p][0:64, msl], qt_sb[p][0:64, lsl],
                )
                nc.tensor.matmul(
                    sc[:, LB:2 * LB],
                    kt_sb[p][64:128, msl], qt_sb[p][64:128, lsl],
                )
                e = epool.tile((128, 2 * LB), BF16, name="e_sb")
                nc.scalar.activation(e[:], sc[:], Exp)
                # A·V per head, [V_h | 1] lhsT: row 64 = denominator
                nc.tensor.matmul(
                    ava[0:65, :],
                    v_sb[:, m, (2 * p) * 65:(2 * p) * 65 + 65],
                    e[:, 0:LB],
                    start=(m == 0), stop=(m == MC - 1),
                )
                nc.tensor.matmul(
                    avb[0:65, :],
                    v_sb[:, m, (2 * p + 1) * 65:(2 * p + 1) * 65 + 65],
                    e[:, LB:2 * LB],
                    start=(m == 0), stop=(m == MC - 1),
                )

            def attn_pair_finish(lb, p, ava, avb):
                # copy raw A·V (+denominator row) to SBUF, freeing the
                # PSUM accumulators immediately; normalize from SBUF
                lsl = slice(lb * LB, (lb + 1) * LB)
                sva = normp.tile((65, LB), F32, name="sva_sb")
                svb = normp.tile((65, LB), F32, name="svb_sb")
                nc.vector.tensor_copy(sva[:], ava[0:65, :])
                nc.vector.tensor_copy(svb[:], avb[0:65, :])
                ra = normp.tile((1, LB), F32, name="ra_sb")
                rb = normp.tile((1, LB), F32, name="rb_sb")
                nc.vector.reciprocal(ra[:], sva[64:65, :])
                nc.vector.reciprocal(rb[:], svb[64:65, :])
                rba = normp.tile((64, LB), F32, name="rba_sb")
                rbb = normp.tile((64, LB), F32, name="rbb_sb")
                nc.gpsimd.partition_broadcast(rba[:], ra[:])
                nc.gpsimd.partition_broadcast(rbb[:], rb[:])
                nc.vector.tensor_tensor(
                    ot_sb[p][0:64, lsl], sva[0:64, :], rba[:], MULT)
                nc.vector.tensor_tensor(
                    ot_sb[p][64:128, lsl], svb[0:64, :], rbb[:], MULT)

            def outproj_tile(lb, lt, ns):
                row0 = lb * LB + lt * 128
                yp = accps.tile((128, 512), F32, name="acc_ps")
                for jc in range(2):
                    nc.tensor.matmul(
                        yp[:],
                        ot_sb[jc][:, row0:row0 + 128],
                        wo_sb[:, jc, ns * 512:(ns + 1) * 512],
                        start=(jc == 0), stop=(jc == 1),
                    )
                ty = ypool.tile((128, 512), F32, name="y_sb")
                nc.any.tensor_copy(ty[:], yp[:])
                nc.sync.dma_start(
                    y_d[row0:row0 + 128, ns * 512:(ns + 1) * 512], ty[:])

            def qk_group(w_sb, b_sb, dst, lb, jc):
                ps = accps.tile((128, LB), F32, name="acc_ps")
                for ko in range(KO):
                    nc.tensor.matmul(
                        ps[:],
                        w_sb[:, ko, jc * 128:(jc + 1) * 128],
                        xt_sb[:, ko, lb * LB:(lb + 1) * LB],
                        start=(ko == 0), stop=(ko == KO - 1),
                    )
                nc.vector.tensor_scalar(
                    dst[jc][:, lb * LB:(lb + 1) * LB], ps[:],
                    b_sb[:, jc:jc + 1], None, ADD,
                )

            # Emission schedule (PE is in-order; exp is slower per chunk
            # than its paired matmuls, so non-attention matmul groups are
            # interleaved as filler to keep PE dense):
            #   Q(lb0); then lb0-pair0's m-chunks fused behind their K/V
            #   blocks; then the remaining (lb, pair) loops with
            #   out-projection(lb-1) and Q(lb+1) as filler.
            proj_qk(wq_sb, bq_sb, qt_sb, 0)
            ava0, avb0 = attn_pair_start()
            for mb in range(NLB):
                qk_group(wk_sb, bk_sb, kt_sb, mb, 0)
                qk_group(wk_sb, bk_sb, kt_sb, mb, 1)
                proj_v(mb)
                for m in range(4 * mb, 4 * mb + 4):
                    attn_chunk(0, 0, m, ava0, avb0)
            attn_pair_finish(0, 0, ava0, avb0)

            filler = []
            for lb in range(NLB):
                for p in range(2):
                    if lb == 0 and p == 0:
                        continue
                    # Q(lb+1) fillers depend only on X^T/WQ (always ready)
                    # and soak the exp backlog right at a pair start; the
                    # out-projection of lb-1 waits for ot(lb-1), which is
                    # safely done one pair later.
                    if p == 0 and lb >= 1 and lb + 1 < NLB:
                        filler += [
                            (lambda lb=lb, jc=jc:
                             qk_group(wq_sb, bq_sb, qt_sb, lb + 1, jc))
                            for jc in range(2)
                        ]
                    if p == 1 and lb == 0:
                        filler += [
                            (lambda jc=jc:
                             qk_group(wq_sb, bq_sb, qt_sb, 1, jc))
                            for jc in range(2)
                        ]
                    if p == 1 and lb >= 1:
                        filler += [
                            (lambda lb=lb, lt=lt, ns=ns:
                             outproj_tile(lb - 1, lt, ns))
                            for lt in range(4) for ns in range(4)
                        ]
                    ava, avb = attn_pair_start()
                    for m in range(MC):
                        if m == 0:
                            # soak the exp backlog from the previous pair's
                            # tail with ~2.5us of independent matmul work
                            budget = 1 if (p == 0 and lb >= 1) else 5
                            for _ in range(min(budget, len(filler))):
                                filler.pop(0)()
                        attn_chunk(lb, p, m, ava, avb)
                        if filler:
                            filler.pop(0)()
                    attn_pair_finish(lb, p, ava, avb)
            for f in filler:
                f()
            for lt in range(4):
                for ns in range(4):
                    outproj_tile(NLB - 1, lt, ns)

    nc.compile()
    return nc


def make_core_inputs(X, Wq_w, Wq_b, Wk_w, Wk_b, Wv_w, Wv_b, Wo_w):
    """Host-side sharding: per-core input dicts (shared XT + per-core weights)."""
    X = np.asarray(X, np.float32)
    bf = ml_dtypes.bfloat16
    xt = np.ascontiguousarray(X.T).astype(bf)
    scale = 1.0 / np.sqrt(np.float32(D))
    in_maps = []
    for c in range(NCORES):
        idx = np.array([d * NH + h for h in range(c * HPC, (c + 1) * HPC)
                        for d in range(DH)], np.int64)

        def kxj(w, s=1.0):
            # (D_in=K, JC) -> (128, KO, JC) with [p, ko, j] = w.T[ko*128+p, j]
            wt = np.ascontiguousarray((np.asarray(w, np.float32)[idx, :] * s).T)
            return np.ascontiguousarray(
                wt.reshape(KO, 128, JC).transpose(1, 0, 2)).astype(bf)

        wo = np.ascontiguousarray(np.asarray(Wo_w, np.float32)[:, idx].T)  # (JC, D)
        wo = np.ascontiguousarray(wo.reshape(2, 128, D).transpose(1, 0, 2)).astype(bf)

        def bcol(b, s=1.0):
            return np.ascontiguousarray(
                (np.asarray(b, np.float32)[idx] * s).reshape(2, 128).T)

        in_maps.append({
            "XT": xt,
            "WQ": kxj(Wq_w, scale), "WK": kxj(Wk_w), "WV": kxj(Wv_w),
            "WO": wo,
            "BQ": bcol(Wq_b, scale), "BK": bcol(Wk_b),
            "BV": np.asarray(Wv_b, np.float32)[idx].reshape(1, JC).astype(bf),
            "ONES": np.ones((1, 128), bf),
        })
    return in_maps


_prog_cache = {}


def kernel(X, Wq_w, Wq_b, Wk_w, Wk_b, Wv_w, Wv_b, Wo_w, Wo_b, _trace=False):
    from concourse.bass_utils import run_bass_kernel_spmd

    if "nc" not in _prog_cache:
        _prog_cache["nc"] = build_program()
    nc = _prog_cache["nc"]
    in_maps = make_core_inputs(X, Wq_w, Wq_b, Wk_w, Wk_b, Wv_w, Wv_b, Wo_w)
    res = run_bass_kernel_spmd(nc, in_maps, core_ids=list(range(NCORES)),
                               trace=_trace)
    y = np.zeros((L, D), np.float64)
    for r in res.results:
        y += r["Y"].astype(np.float64)
    y += np.asarray(Wo_b, np.float32).astype(np.float64)
    out = y.astype(np.float32)
    if _trace:
        kernel.last_results = res
    return out


# revision 32
# speedup vs baseline: 1.0529x; 1.0529x over previous
"""Trainium2 Bass kernel for nn_MultiHeadAttention_62319975465542.

Tensor-parallel over heads (Megatron-style): 32 heads sharded 4-per-core
across 8 NeuronCores. Each core computes its heads' QKV projections,
attention, and a partial output projection; the host sums the 8 partials
(the all-reduce after Wo) and adds the output bias.

Reference layout note: Q = (X @ Wq.T + b).reshape(L, D_HEAD, NUM_HEADS),
so head h owns interleaved feature columns {d*32 + h : d in 0..63}. The
host pre-gathers those columns into contiguous per-core blocks.

v3: all matmul inputs in bf16 (fast weight load, halved DMA), X^T fully
SBUF-resident, and emission ordered so ScalarE exp for one l-block
overlaps TensorE work of the next (Q(lb0) first, then K/V, then
attention blocks). Softmax denominators ride in the A-V matmul as a
17th "ones" V-column ([V_h | 1], M=65); normalization is reciprocal +
gpsimd partition-broadcast + one vector multiply. PSUM stays fp32.
"""

import numpy as np
import ml_dtypes

import concourse.bass as bass
import concourse.tile as tile
import concourse.mybir as mybir
from concourse import bacc

F32 = mybir.dt.float32
BF16 = mybir.dt.bfloat16
Identity = mybir.ActivationFunctionType.Identity
Exp = mybir.ActivationFunctionType.Exp
MULT = mybir.AluOpType.mult
ADD = mybir.AluOpType.add

L = 2048          # sequence length
D = 2048          # d_model
NH = 32           # total heads
DH = 64           # head dim
NCORES = 8
HPC = NH // NCORES   # heads per core = 4
JC = HPC * DH        # per-core projected width = 256
LB = 512             # l-block width
NLB = L // LB        # 4
KO = D // 128        # 16 contraction chunks
MC = L // 128        # 16 key chunks


def build_program():
    nc = bacc.Bacc("TRN2", target_bir_lowering=False, debug=False)

    xt_d = nc.dram_tensor("XT", (D, L), BF16, kind="ExternalInput")
    wq_d = nc.dram_tensor("WQ", (128, KO, JC), BF16, kind="ExternalInput")
    wk_d = nc.dram_tensor("WK", (128, KO, JC), BF16, kind="ExternalInput")
    wv_d = nc.dram_tensor("WV", (128, KO, JC), BF16, kind="ExternalInput")
    wo_d = nc.dram_tensor("WO", (128, 2, D), BF16, kind="ExternalInput")
    bq_d = nc.dram_tensor("BQ", (128, 2), F32, kind="ExternalInput")
    bk_d = nc.dram_tensor("BK", (128, 2), F32, kind="ExternalInput")
    bv_d = nc.dram_tensor("BV", (1, JC), BF16, kind="ExternalInput")
    ones_d = nc.dram_tensor("ONES", (1, 128), BF16, kind="ExternalInput")
    y_d = nc.dram_tensor("Y", (L, D), F32, kind="ExternalOutput")

    with tile.TileContext(nc) as tc, nc.allow_low_precision(
            reason="bf16 activations are within tolerance for this op"):
        with (
            tc.tile_pool(name="const", bufs=1) as cp,
            tc.tile_pool(name="epool", bufs=20) as epool,
            tc.tile_pool(name="norm", bufs=2) as normp,
            tc.tile_pool(name="ysb", bufs=3) as ypool,
            tc.tile_pool(name="scps", bufs=2, space="PSUM") as scps,
            tc.tile_pool(name="accps", bufs=4, space="PSUM") as accps,
        ):
            wq_sb = cp.tile((128, KO, JC), BF16)
            wk_sb = cp.tile((128, KO, JC), BF16)
            wv_sb = cp.tile((128, KO, JC), BF16)
            wo_sb = cp.tile((128, 2, D), BF16)
            bq_sb = cp.tile((128, 2), F32)
            bk_sb = cp.tile((128, 2), F32)
            bv_sb = cp.tile((1, JC), BF16)
            ones_sb = cp.tile((1, 128), BF16)
            xt_sb = cp.tile((128, KO, L), BF16)
            nc.sync.dma_start(wq_sb[:], wq_d[:])
            nc.sync.dma_start(wk_sb[:], wk_d[:])
            nc.sync.dma_start(wv_sb[:], wv_d[:])
            nc.sync.dma_start(wo_sb[:], wo_d[:])
            nc.sync.dma_start(bq_sb[:], bq_d[:])
            nc.sync.dma_start(bk_sb[:], bk_d[:])
            nc.sync.dma_start(bv_sb[:], bv_d[:])
            nc.sync.dma_start(ones_sb[:], ones_d[:])

            qt_sb = [cp.tile((128, L), BF16, name=f"qt{p}") for p in range(2)]
            kt_sb = [cp.tile((128, L), BF16, name=f"kt{p}") for p in range(2)]
            v_sb = cp.tile((128, MC, HPC * 65), BF16)
            nc.vector.memset(v_sb[:], 1.0)
            ot_sb = [cp.tile((128, L), BF16, name=f"ot{p}") for p in range(2)]

            # X^T loads: lb-major so Q(lb0)/K(lb0) unblock after 16 DMAs
            for lb in range(NLB):
                for ko in range(KO):
                    nc.sync.dma_start(
                        xt_sb[:, ko, lb * LB:(lb + 1) * LB],
                        xt_d[ko * 128:(ko + 1) * 128, lb * LB:(lb + 1) * LB],
                    )

            def proj_qk(w_sb, b_sb, dst, lb):
                """(j, l) layout projection with per-partition bias via DVE."""
                for jc in range(2):
                    ps = accps.tile((128, LB), F32, name="acc_ps")
                    for ko in range(KO):
                        nc.tensor.matmul(
                            ps[:],
                            w_sb[:, ko, jc * 128:(jc + 1) * 128],
                            xt_sb[:, ko, lb * LB:(lb + 1) * LB],
                            start=(ko == 0), stop=(ko == KO - 1),
                        )
                    nc.vector.tensor_scalar(
                        dst[jc][:, lb * LB:(lb + 1) * LB], ps[:],
                        b_sb[:, jc:jc + 1], None, ADD,
                    )

            def proj_v(lb):
                """V in (l, j) layout; bias via K=1 ones-row matmul."""
                for lt in range(4):
                    vp = accps.tile((128, LB), F32, name="acc_ps")
                    for ko in range(KO):
                        nc.tensor.matmul(
                            vp[:, 0:JC],
                            xt_sb[:, ko, lb * LB + lt * 128:lb * LB + (lt + 1) * 128],
                            wv_sb[:, ko, :],
                            start=(ko == 0), stop=False,
                        )
                    nc.tensor.matmul(
                        vp[:, 0:JC], ones_sb[0:1, :], bv_sb[0:1, :],
                        start=False, stop=True,
                    )
                    nc.vector.tensor_copy(
                        v_sb[:, lb * 4 + lt, :].rearrange(
                            "p (h e) -> p h e", h=HPC)[:, :, 0:DH],
                        vp[:, 0:JC].rearrange("p (h d) -> p h d", h=HPC))

            def attn_pair_start():
                ava = accps.tile((128, LB), F32, name="acc_ps")
                avb = accps.tile((128, LB), F32, name="acc_ps")
                return ava, avb

            def score_chunk(lb, p, m):
                """Scores + exp for one (m, l-block) chunk; returns the
                bf16 exp tile (kept in SBUF for the later A·V phase)."""
                lsl = slice(lb * LB, (lb + 1) * LB)
                msl = slice(m * 128, (m + 1) * 128)
                sc = scps.tile((128, 2 * LB), F32, name="sc_ps")
                nc.tensor.matmul(
                    sc[:, 0:LB],
                    kt_sb[p][0:64, msl], qt_sb[p][0:64, lsl],
                )
                nc.tensor.matmul(
                    sc[:, LB:2 * LB],
                    kt_sb[p][64:128, msl], qt_sb[p][64:128, lsl],
                )
                e = epool.tile((128, 2 * LB), BF16, name="e_sb")
                nc.scalar.activation(e[:], sc[:], Exp)
                return e

            def av_chunk(p, m, ava, avb, e):
                # A·V per head, [V_h | 1] lhsT: row 64 = denominator
                nc.tensor.matmul(
                    ava[0:65, :],
                    v_sb[:, m, (2 * p) * 65:(2 * p) * 65 + 65],
                    e[:, 0:LB],
                    start=(m == 0), stop=(m == MC - 1),
                )
                nc.tensor.matmul(
                    avb[0:65, :],
                    v_sb[:, m, (2 * p + 1) * 65:(2 * p + 1) * 65 + 65],
                    e[:, LB:2 * LB],
                    start=(m == 0), stop=(m == MC - 1),
                )

            def attn_chunk(lb, p, m, ava, avb):
                e = score_chunk(lb, p, m)
                av_chunk(p, m, ava, avb, e)

            def attn_pair_finish(lb, p, ava, avb):
                # copy raw A·V (+denominator row) to SBUF, freeing the
                # PSUM accumulators immediately; normalize from SBUF
                lsl = slice(lb * LB, (lb + 1) * LB)
                sva = normp.tile((65, LB), F32, name="sva_sb")
                svb = normp.tile((65, LB), F32, name="svb_sb")
                nc.vector.tensor_copy(sva[:], ava[0:65, :])
                nc.vector.tensor_copy(svb[:], avb[0:65, :])
                ra = normp.tile((1, LB), F32, name="ra_sb")
                rb = normp.tile((1, LB), F32, name="rb_sb")
                nc.vector.reciprocal(ra[:], sva[64:65, :])
                nc.vector.reciprocal(rb[:], svb[64:65, :])
                rba = normp.tile((64, LB), F32, name="rba_sb")
                rbb = normp.tile((64, LB), F32, name="rbb_sb")
                nc.gpsimd.partition_broadcast(rba[:], ra[:])
                nc.gpsimd.partition_broadcast(rbb[:], rb[:])
                nc.vector.tensor_tensor(
                    ot_sb[p][0:64, lsl], sva[0:64, :], rba[:], MULT)
                nc.vector.tensor_tensor(
                    ot_sb[p][64:128, lsl], svb[0:64, :], rbb[:], MULT)

            def outproj_tile(lb, lt, ns):
                row0 = lb * LB + lt * 128
                yp = accps.tile((128, 512), F32, name="acc_ps")
                for jc in range(2):
                    nc.tensor.matmul(
                        yp[:],
                        ot_sb[jc][:, row0:row0 + 128],
                        wo_sb[:, jc, ns * 512:(ns + 1) * 512],
                        start=(jc == 0), stop=(jc == 1),
                    )
                ty = ypool.tile((128, 512), F32, name="y_sb")
                nc.any.tensor_copy(ty[:], yp[:])
                nc.sync.dma_start(
                    y_d[row0:row0 + 128, ns * 512:(ns + 1) * 512], ty[:])

            def qk_group(w_sb, b_sb, dst, lb, jc):
                ps = accps.tile((128, LB), F32, name="acc_ps")
                for ko in range(KO):
                    nc.tensor.matmul(
                        ps[:],
                        w_sb[:, ko, jc * 128:(jc + 1) * 128],
                        xt_sb[:, ko, lb * LB:(lb + 1) * LB],
                        start=(ko == 0), stop=(ko == KO - 1),
                    )
                nc.vector.tensor_scalar(
                    dst[jc][:, lb * LB:(lb + 1) * LB], ps[:],
                    b_sb[:, jc:jc + 1], None, ADD,
                )

            # Emission schedule. Streams S_i = (lb, p) pairs. The scores+
            # exp phase of stream i is e-buffered in SBUF and overlaps the
            # A·V phase of stream i-1 on PE, so A·V never waits on exp.
            # S0=(0,0) runs fused inside the K/V phase; S1=(0,1)'s scores
            # also run there. Q/out-projection groups act as filler.
            proj_qk(wq_sb, bq_sb, qt_sb, 0)
            ava0, avb0 = attn_pair_start()
            e1 = []
            for mb in range(NLB):
                qk_group(wk_sb, bk_sb, kt_sb, mb, 0)
                qk_group(wk_sb, bk_sb, kt_sb, mb, 1)
                proj_v(mb)
                for m in range(4 * mb, 4 * mb + 4):
                    attn_chunk(0, 0, m, ava0, avb0)
                    e1.append(score_chunk(0, 1, m))
            attn_pair_finish(0, 0, ava0, avb0)

            # Q(lb1) must be resident before stream (1,0)'s scores
            proj_qk(wq_sb, bq_sb, qt_sb, 1)

            filler = [
                (lambda lb=lb, jc=jc: qk_group(wq_sb, bq_sb, qt_sb, lb, jc))
                for lb in range(2, NLB) for jc in range(2)
            ]
            prev = (0, 1, e1)    # stream whose A·V phase is pending
            for lb, p in [(lb, p) for lb in range(1, NLB) for p in range(2)]:
                e_list = []
                pava, pavb = attn_pair_start()
                for m in range(MC):
                    e_list.append(score_chunk(lb, p, m))
                    av_chunk(prev[1], m, pava, pavb, prev[2][m])
                    if filler and (m % 2 == 0):
                        filler.pop(0)()
                attn_pair_finish(prev[0], prev[1], pava, pavb)
                if prev[1] == 1:
                    flb = prev[0]
                    filler += [
                        (lambda flb=flb, lt=lt, ns=ns:
                         outproj_tile(flb, lt, ns))
                        for lt in range(4) for ns in range(4)
                    ]
                prev = (lb, p, e_list)
            # tail: A·V of the last stream, its normalize, remaining fill,
            # and the final out-projection
            pava, pavb = attn_pair_start()
            for m in range(MC):
                av_chunk(prev[1], m, pava, pavb, prev[2][m])
                if filler:
                    filler.pop(0)()
            attn_pair_finish(prev[0], prev[1], pava, pavb)
            for f in filler:
                f()
            for lt in range(4):
                for ns in range(4):
                    outproj_tile(NLB - 1, lt, ns)

    nc.compile()
    return nc


def make_core_inputs(X, Wq_w, Wq_b, Wk_w, Wk_b, Wv_w, Wv_b, Wo_w):
    """Host-side sharding: per-core input dicts (shared XT + per-core weights)."""
    X = np.asarray(X, np.float32)
    bf = ml_dtypes.bfloat16
    xt = np.ascontiguousarray(X.T).astype(bf)
    scale = 1.0 / np.sqrt(np.float32(D))
    in_maps = []
    for c in range(NCORES):
        idx = np.array([d * NH + h for h in range(c * HPC, (c + 1) * HPC)
                        for d in range(DH)], np.int64)

        def kxj(w, s=1.0):
            # (D_in=K, JC) -> (128, KO, JC) with [p, ko, j] = w.T[ko*128+p, j]
            wt = np.ascontiguousarray((np.asarray(w, np.float32)[idx, :] * s).T)
            return np.ascontiguousarray(
                wt.reshape(KO, 128, JC).transpose(1, 0, 2)).astype(bf)

        wo = np.ascontiguousarray(np.asarray(Wo_w, np.float32)[:, idx].T)  # (JC, D)
        wo = np.ascontiguousarray(wo.reshape(2, 128, D).transpose(1, 0, 2)).astype(bf)

        def bcol(b, s=1.0):
            return np.ascontiguousarray(
                (np.asarray(b, np.float32)[idx] * s).reshape(2, 128).T)

        in_maps.append({
            "XT": xt,
            "WQ": kxj(Wq_w, scale), "WK": kxj(Wk_w), "WV": kxj(Wv_w),
            "WO": wo,
            "BQ": bcol(Wq_b, scale), "BK": bcol(Wk_b),
            "BV": np.asarray(Wv_b, np.float32)[idx].reshape(1, JC).astype(bf),
            "ONES": np.ones((1, 128), bf),
        })
    return in_maps


_prog_cache = {}


def kernel(X, Wq_w, Wq_b, Wk_w, Wk_b, Wv_w, Wv_b, Wo_w, Wo_b, _trace=False):
    from concourse.bass_utils import run_bass_kernel_spmd

    if "nc" not in _prog_cache:
        _prog_cache["nc"] = build_program()
    nc = _prog_cache["nc"]
    in_maps = make_core_inputs(X, Wq_w, Wq_b, Wk_w, Wk_b, Wv_w, Wv_b, Wo_w)
    res = run_bass_kernel_spmd(nc, in_maps, core_ids=list(range(NCORES)),
                               trace=_trace)
    y = np.zeros((L, D), np.float64)
    for r in res.results:
        y += r["Y"].astype(np.float64)
    y += np.asarray(Wo_b, np.float32).astype(np.float64)
    out = y.astype(np.float32)
    if _trace:
        kernel.last_results = res
    return out
